# revision 2
# baseline (speedup 1.0000x reference)
"""Mat2Twist Trainium2 kernel: batch of 3x3 rotation matrices -> twist vectors.

For each matrix R:  tr = trace(R); x = (tr-1)/2 = cos(theta)
  theta = arccos(x) = pi/2 - arctan(x / sqrt(1 - x^2))
  2*sin(theta) = 2*sqrt(1 - x^2)
  w = [R21-R12, R02-R20, R10-R01]   (unnormalized axis, |w| = 2 sin theta)
  out = theta * w / (2 sin theta) = (pi/4 - arctan(x*r)/2) * r * w,
        r = 1/sqrt(1-x^2)

Data-parallel over 8 NeuronCores; each core streams its shard through
SBUF in [128, 9*M] tiles (M matrices per partition per tile).
"""

import numpy as np

import concourse.bass as bass
import concourse.mybir as mybir
from concourse.tile import TileContext
from concourse.bass_utils import run_bass_kernel_spmd

B = 4194304
NCORES = 8
P = 128
M = 1024               # matrices per partition per tile
N_C = B // NCORES      # 524288 matrices per core
T = N_C // (P * M)     # 4 tiles per core

F32 = mybir.dt.float32
ACT = mybir.ActivationFunctionType
PI_4 = float(np.pi / 4.0)


def _split_multi_waits(nc):
    """This container's walrus build rejects >1 sem-wait per instruction
    ("Too many sync wait commands"); hoist extras onto preceding NOPs."""
    for f in nc.m.functions:
        for blk in f.blocks:
            il = blk.instructions
            new = []
            for ins in il:
                si = ins.sync_info
                if si is not None and si.on_wait is not None and len(si.on_wait) > 1:
                    waits = list(si.on_wait)
                    for j, w in enumerate(waits[:-1]):
                        nop = mybir.InstNoOp(name=f"{ins.name}-ws{j}", engine=ins.engine)
                        nop.sync_info = mybir.SyncInfo(on_wait=[w], on_update=[])
                        new.append(nop)
                    ins.sync_info = mybir.SyncInfo(
                        on_wait=[waits[-1]], on_update=list(si.on_update or [])
                    )
                new.append(ins)
            il[:] = new


def _build_kernel():
    nc = bass.Bass()
    x_in = nc.dram_tensor("mat_in", [T, P, 9 * M], F32, kind="ExternalInput")
    y_out = nc.dram_tensor("twist_out", [T, P, 3 * M], F32, kind="ExternalOutput")

    with TileContext(nc) as tc:
        with tc.tile_pool(name="io", bufs=2) as io_pool, \
             tc.tile_pool(name="tmp", bufs=2) as tmp:
            for ti in range(T):
                tile = io_pool.tile([P, 9 * M], F32, tag="in")
                nc.sync.dma_start(out=tile, in_=x_in[ti])
                t3 = tile.rearrange("p (j k) -> p k j", k=9)  # [P, 9, M] strided view

                ot = io_pool.tile([P, 3 * M], F32, tag="out")
                o3 = ot.rearrange("p (j k) -> p k j", k=3)
                # w components into the output tile (scaled in place later)
                for k, (a, b) in enumerate(((7, 5), (2, 6), (3, 1))):
                    nc.vector.tensor_sub(out=o3[:, k], in0=t3[:, a], in1=t3[:, b])

                tr = tmp.tile([P, M], F32, tag="tr")
                nc.vector.tensor_add(out=tr, in0=t3[:, 0], in1=t3[:, 4])
                nc.vector.tensor_add(out=tr, in0=tr, in1=t3[:, 8])

                x = tmp.tile([P, M], F32, tag="x")
                nc.scalar.activation(x, tr, ACT.Copy, bias=-0.5, scale=0.5)
                v = tmp.tile([P, M], F32, tag="v")
                nc.scalar.activation(v, x, ACT.Square)
                # r = 1/sin(theta) = exp(-0.5*ln(1-x^2)); Ln and Exp share one
                # ACT table set (natural_log_exp_and_others).
                lg = tmp.tile([P, M], F32, tag="lg")
                nc.scalar.activation(lg, v, ACT.Ln, bias=1.0, scale=-1.0)
                r = tmp.tile([P, M], F32, tag="r")
                nc.scalar.activation(r, lg, ACT.Exp, scale=-0.5)

                xr = tmp.tile([P, M], F32, tag="xr")  # x/sin = cot(theta)
                nc.vector.tensor_mul(out=xr, in0=x, in1=r)

                t_at = tmp.tile([P, M], F32, tag="t_at")
                nc.scalar.activation(t_at, xr, ACT.Arctan)

                # scale = (pi/4 - arctan/2) * r = theta/(2 sin theta)
                g = tmp.tile([P, M], F32, tag="g")
                nc.vector.tensor_scalar(
                    out=g, in0=t_at, scalar1=-0.5, scalar2=PI_4,
                    op0=mybir.AluOpType.mult, op1=mybir.AluOpType.add,
                )
                sc = tmp.tile([P, M], F32, tag="sc")
                nc.vector.tensor_mul(out=sc, in0=g, in1=r)

                for k in range(3):
                    nc.vector.tensor_mul(out=o3[:, k], in0=sc, in1=o3[:, k])
                nc.sync.dma_start(out=y_out[ti], in_=ot)

    _split_multi_waits(nc)
    return nc


_NC_CACHE = []


def kernel(mat_batch: np.ndarray) -> np.ndarray:
    if not _NC_CACHE:
        _NC_CACHE.append(_build_kernel())
    nc = _NC_CACHE[0]

    flat = np.ascontiguousarray(mat_batch, dtype=np.float32).reshape(
        NCORES, T, P, 9 * M
    )
    in_maps = [{"mat_in": flat[i]} for i in range(NCORES)]
    res = run_bass_kernel_spmd(nc, in_maps, core_ids=list(range(NCORES)))
    return np.concatenate(
        [r["twist_out"].reshape(N_C, 3) for r in res.results], axis=0
    )


# revision 10
# speedup vs baseline: 9.1575x; 9.1575x over previous
"""Mat2Twist Trainium2 kernel: batch of 3x3 rotation matrices -> twist vectors.

For each matrix R:  tr = trace(R); x = (tr-1)/2 = cos(theta)
  theta = arccos(x) = pi/2 - arctan(x / sqrt(1 - x^2))
  2*sin(theta) = 2*sqrt(1 - x^2)
  w = [R21-R12, R02-R20, R10-R01]   (unnormalized axis, |w| = 2 sin theta)
  out = theta * w / (2 sin theta) = (pi/4 - arctan(x*r)/2) * r * w,
        r = 1/sqrt(1-x^2) = exp(-0.5*ln(1-x^2))

Data-parallel over 8 NeuronCores. The host pre-arranges each core's
shard tile-major/component-major: chunk ci covers MS[ci] matrices per
partition, and within a partition-row the 9 components are stored as
contiguous blocks in PERM order, so every on-chip vector op and every
DMA is unit-stride:
  w  = chunk[0:3m] - chunk[3m:6m]      (one fused subtract, 3m wide)
  tr = chunk[6m:7m]+chunk[7m:8m]+chunk[8m:9m]
Output is produced component-major per chunk and re-interleaved on host.

Chunk sizes are asymmetric (small first/last) to shorten pipeline
fill/drain, and chunks are processed in pairs with a forced ordering of
ACT ops so the activation table set switches once per chunk instead of
twice (natural_log_exp <-> trig).
"""

import numpy as np

import concourse.bass as bass
import concourse.mybir as mybir
from concourse.tile import TileContext
from concourse.tile_rust import add_dep_helper
from concourse.bass_utils import run_bass_kernel_spmd

B = 4194304
NCORES = 8
P = 128
N_C = B // NCORES        # 524288 matrices per core
MPP = N_C // P           # 4096 matrices per partition
MS = [512, 1024, 1024, 1024, 512]   # per-chunk matrices per partition
assert sum(MS) == MPP

# component order in DRAM (flat 3x3 index): minuends, subtrahends, diagonal
PERM = [7, 2, 3, 5, 6, 1, 0, 4, 8]

F32 = mybir.dt.float32
ACT = mybir.ActivationFunctionType
PI_4 = float(np.pi / 4.0)
MAXM = max(MS)


def _split_multi_waits(nc):
    """This container's walrus build rejects >1 sem-wait per instruction
    ("Too many sync wait commands"); hoist extras onto preceding NOPs."""
    for f in nc.m.functions:
        for blk in f.blocks:
            il = blk.instructions
            new = []
            for ins in il:
                si = ins.sync_info
                if si is not None and si.on_wait is not None and len(si.on_wait) > 1:
                    waits = list(si.on_wait)
                    for j, w in enumerate(waits[:-1]):
                        nop = mybir.InstNoOp(name=f"{ins.name}-ws{j}", engine=ins.engine)
                        nop.sync_info = mybir.SyncInfo(on_wait=[w], on_update=[])
                        new.append(nop)
                    ins.sync_info = mybir.SyncInfo(
                        on_wait=[waits[-1]], on_update=list(si.on_update or [])
                    )
                new.append(ins)
            il[:] = new


def _build_kernel():
    nc = bass.Bass()
    # flat per-core buffers; chunk ci occupies rows [off*P*9 ...] tile-major
    x_in = nc.dram_tensor("mat_in", [N_C * 9], F32, kind="ExternalInput")
    y_out = nc.dram_tensor("twist_out", [N_C * 3], F32, kind="ExternalOutput")

    with TileContext(nc) as tc:
        with tc.tile_pool(name="io", bufs=2) as io_pool, \
             tc.tile_pool(name="io_out", bufs=3) as oo_pool, \
             tc.tile_pool(name="tmp", bufs=2) as tmp:

            def stage1(ci, off, m):
                tile = io_pool.tile([P, 9 * MAXM], F32, tag="in", name=f"in{ci}")[:, : 9 * m]
                src = x_in[off * P * 9 : (off + m) * P * 9].rearrange(
                    "(p n) -> p n", p=P
                )
                nc.sync.dma_start(out=tile, in_=src)

                ot = oo_pool.tile([P, 3 * MAXM], F32, tag="out", name=f"out{ci}")[:, : 3 * m]
                nc.vector.tensor_sub(
                    out=ot, in0=tile[:, 0 : 3 * m], in1=tile[:, 3 * m : 6 * m]
                )

                tr = tmp.tile([P, MAXM], F32, tag="tr", name=f"tr{ci}")[:, :m]
                nc.vector.tensor_add(
                    out=tr, in0=tile[:, 6 * m : 7 * m], in1=tile[:, 7 * m : 8 * m]
                )
                nc.vector.tensor_add(out=tr, in0=tr, in1=tile[:, 8 * m : 9 * m])

                x = tmp.tile([P, MAXM], F32, tag="x", name=f"x{ci}")[:, :m]
                nc.scalar.activation(x, tr, ACT.Copy, bias=-0.5, scale=0.5)
                v = tmp.tile([P, MAXM], F32, tag="v", name=f"v{ci}")[:, :m]
                nc.scalar.activation(v, x, ACT.Square)
                lg = tmp.tile([P, MAXM], F32, tag="lg", name=f"lg{ci}")[:, :m]
                nc.scalar.activation(lg, v, ACT.Ln, bias=1.0, scale=-1.0)
                r = tmp.tile([P, MAXM], F32, tag="r", name=f"r{ci}")[:, :m]  # 1/sin(theta)
                i_exp = nc.scalar.activation(r, lg, ACT.Exp, scale=-0.5)

                xr = tmp.tile([P, MAXM], F32, tag="xr", name=f"xr{ci}")[:, :m]  # cot(theta)
                nc.vector.tensor_mul(out=xr, in0=x, in1=r)
                return ot, r, xr, i_exp

            def stage2(ci, off, m, ot, r, xr):
                t_at = tmp.tile([P, MAXM], F32, tag="t_at", name=f"t_at{ci}")[:, :m]
                i_at = nc.scalar.activation(t_at, xr, ACT.Arctan)

                g = tmp.tile([P, MAXM], F32, tag="g", name=f"g{ci}")[:, :m]
                nc.vector.tensor_scalar(
                    out=g, in0=t_at, scalar1=-0.5, scalar2=PI_4,
                    op0=mybir.AluOpType.mult, op1=mybir.AluOpType.add,
                )
                sc = tmp.tile([P, MAXM], F32, tag="sc", name=f"sc{ci}")[:, :m]
                nc.vector.tensor_mul(out=sc, in0=g, in1=r)

                for k in range(3):
                    blk = ot[:, k * m : (k + 1) * m]
                    nc.vector.tensor_mul(out=blk, in0=sc, in1=blk)
                dst = y_out[off * P * 3 : (off + m) * P * 3].rearrange(
                    "(p n) -> p n", p=P
                )
                nc.sync.dma_start(out=dst, in_=ot)
                return i_at

            offs = np.concatenate([[0], np.cumsum(MS)[:-1]])
            for cj in range(len(MS)):
                ot, r, xr, _ = stage1(cj, int(offs[cj]), MS[cj])
                stage2(cj, int(offs[cj]), MS[cj], ot, r, xr)

    _split_multi_waits(nc)
    return nc


_NC_CACHE = []


def _host_pack(mat_batch: np.ndarray) -> np.ndarray:
    """[B,3,3] -> [NCORES, N_C*9] tile-major/component-major PERM layout."""
    flat = np.ascontiguousarray(mat_batch, dtype=np.float32).reshape(
        NCORES, N_C, 9
    )
    out = np.empty((NCORES, N_C * 9), np.float32)
    pos = 0
    for m, off in zip(MS, np.concatenate([[0], np.cumsum(MS)[:-1]])):
        off = int(off)
        # chunk: matrices [off*P, (off+m)*P) viewed [P, m, 9] ->  [P, 9, m]
        chunk = flat[:, off * P : (off + m) * P, :].reshape(NCORES, P, m, 9)
        sz = P * m * 9
        out[:, pos : pos + sz] = (
            chunk.transpose(0, 1, 3, 2)[:, :, PERM, :].reshape(NCORES, sz)
        )
        pos += sz
    return out


def _host_unpack(res_list) -> np.ndarray:
    out = np.empty((B, 3), np.float32)
    o = out.reshape(NCORES, N_C, 3)
    for i, r in enumerate(res_list):
        y = r["twist_out"]
        pos = 0
        for m, off in zip(MS, np.concatenate([[0], np.cumsum(MS)[:-1]])):
            off = int(off)
            sz = P * m * 3
            blk = y[pos : pos + sz].reshape(P, 3, m)
            o[i, off * P : (off + m) * P, :] = blk.transpose(0, 2, 1).reshape(
                P * m, 3
            )
            pos += sz
    return out


def kernel(mat_batch: np.ndarray) -> np.ndarray:
    if not _NC_CACHE:
        _NC_CACHE.append(_build_kernel())
    nc = _NC_CACHE[0]

    packed = _host_pack(mat_batch)
    in_maps = [{"mat_in": packed[i]} for i in range(NCORES)]
    res = run_bass_kernel_spmd(nc, in_maps, core_ids=list(range(NCORES)))
    return _host_unpack(res.results)


# revision 13
# speedup vs baseline: 10.8229x; 1.1819x over previous
"""Mat2Twist Trainium2 kernel: batch of 3x3 rotation matrices -> twist vectors.

For each matrix R:  tr = trace(R); x = (tr-1)/2 = cos(theta)
  theta = arccos(x) = pi/2 - arctan(x / sqrt(1 - x^2))
  2*sin(theta) = 2*sqrt(1 - x^2)
  w = [R21-R12, R02-R20, R10-R01]   (unnormalized axis, |w| = 2 sin theta)
  out = theta * w / (2 sin theta) = (pi/4 - arctan(x*r)/2) * r * w,
        r = 1/sqrt(1-x^2) = exp(-0.5*ln(1-x^2))

Data-parallel over 8 NeuronCores. The host pre-arranges each core's
shard tile-major/component-major: chunk ci covers MS[ci] matrices per
partition, and within a partition-row the 9 components are stored as
contiguous blocks in PERM order, so every on-chip vector op and every
DMA is unit-stride:
  w  = chunk[0:3m] - chunk[3m:6m]      (one fused subtract, 3m wide)
  tr = chunk[6m:7m]+chunk[7m:8m]+chunk[8m:9m]
Output is produced component-major per chunk and re-interleaved on host.

Chunk sizes are asymmetric (small first/last) to shorten pipeline
fill/drain.
"""

import numpy as np

import concourse.bass as bass
import concourse.mybir as mybir
from concourse.tile import TileContext
from concourse.bass_utils import run_bass_kernel_spmd

B = 4194304
NCORES = 8
P = 128
N_C = B // NCORES        # 524288 matrices per core
MPP = N_C // P           # 4096 matrices per partition
MS = [512, 1024, 1024, 1024, 512]   # per-chunk matrices per partition
assert sum(MS) == MPP

# component order in DRAM (flat 3x3 index): minuends, subtrahends, diagonal
PERM = [7, 2, 3, 5, 6, 1, 0, 4, 8]

F32 = mybir.dt.float32
ACT = mybir.ActivationFunctionType
PI_4 = float(np.pi / 4.0)
MAXM = max(MS)


def _split_multi_waits(nc):
    """This container's walrus build rejects >1 sem-wait per instruction
    ("Too many sync wait commands"); hoist extras onto preceding NOPs."""
    for f in nc.m.functions:
        for blk in f.blocks:
            il = blk.instructions
            new = []
            for ins in il:
                si = ins.sync_info
                if si is not None and si.on_wait is not None and len(si.on_wait) > 1:
                    waits = list(si.on_wait)
                    for j, w in enumerate(waits[:-1]):
                        nop = mybir.InstNoOp(name=f"{ins.name}-ws{j}", engine=ins.engine)
                        nop.sync_info = mybir.SyncInfo(on_wait=[w], on_update=[])
                        new.append(nop)
                    ins.sync_info = mybir.SyncInfo(
                        on_wait=[waits[-1]], on_update=list(si.on_update or [])
                    )
                new.append(ins)
            il[:] = new


def _build_kernel():
    nc = bass.Bass()
    # flat per-core buffers; chunk ci occupies rows [off*P*9 ...] tile-major
    x_in = nc.dram_tensor("mat_in", [N_C * 9], F32, kind="ExternalInput")
    y_out = nc.dram_tensor("twist_out", [N_C * 3], F32, kind="ExternalOutput")

    with TileContext(nc) as tc:
        with tc.tile_pool(name="io", bufs=2) as io_pool, \
             tc.tile_pool(name="io_out", bufs=3) as oo_pool, \
             tc.tile_pool(name="tmp", bufs=2) as tmp:

            def stage1(ci, off, m):
                tile = io_pool.tile([P, 9 * MAXM], F32, tag="in", name=f"in{ci}")[:, : 9 * m]
                src = x_in[off * P * 9 : (off + m) * P * 9].rearrange(
                    "(p n) -> p n", p=P
                )
                nc.sync.dma_start(out=tile, in_=src)

                ot = oo_pool.tile([P, 3 * MAXM], F32, tag="out", name=f"out{ci}")[:, : 3 * m]
                nc.vector.tensor_sub(
                    out=ot, in0=tile[:, 0 : 3 * m], in1=tile[:, 3 * m : 6 * m]
                )

                tr = tmp.tile([P, MAXM], F32, tag="tr", name=f"tr{ci}")[:, :m]
                nc.vector.tensor_add(
                    out=tr, in0=tile[:, 6 * m : 7 * m], in1=tile[:, 7 * m : 8 * m]
                )
                nc.vector.tensor_add(out=tr, in0=tr, in1=tile[:, 8 * m : 9 * m])

                x = tmp.tile([P, MAXM], F32, tag="x", name=f"x{ci}")[:, :m]
                nc.scalar.activation(x, tr, ACT.Copy, bias=-0.5, scale=0.5)
                v = tmp.tile([P, MAXM], F32, tag="v", name=f"v{ci}")[:, :m]
                nc.scalar.activation(v, x, ACT.Square)
                lg = tmp.tile([P, MAXM], F32, tag="lg", name=f"lg{ci}")[:, :m]
                nc.scalar.activation(lg, v, ACT.Ln, bias=1.0, scale=-1.0)
                r = tmp.tile([P, MAXM], F32, tag="r", name=f"r{ci}")[:, :m]  # 1/sin(theta)
                i_exp = nc.scalar.activation(r, lg, ACT.Exp, scale=-0.5)

                xr = tmp.tile([P, MAXM], F32, tag="xr", name=f"xr{ci}")[:, :m]  # cot(theta)
                nc.vector.tensor_mul(out=xr, in0=x, in1=r)
                return ot, r, xr, i_exp

            def stage2(ci, off, m, ot, r, xr):
                t_at = tmp.tile([P, MAXM], F32, tag="t_at", name=f"t_at{ci}")[:, :m]
                i_at = nc.scalar.activation(t_at, xr, ACT.Arctan)

                g = tmp.tile([P, MAXM], F32, tag="g", name=f"g{ci}")[:, :m]
                nc.vector.tensor_scalar(
                    out=g, in0=t_at, scalar1=-0.5, scalar2=PI_4,
                    op0=mybir.AluOpType.mult, op1=mybir.AluOpType.add,
                )
                sc = tmp.tile([P, MAXM], F32, tag="sc", name=f"sc{ci}")[:, :m]
                nc.vector.tensor_mul(out=sc, in0=g, in1=r)

                for k in range(3):
                    blk = ot[:, k * m : (k + 1) * m]
                    nc.vector.tensor_mul(out=blk, in0=sc, in1=blk)
                dst = y_out[off * P * 3 : (off + m) * P * 3].rearrange(
                    "(p n) -> p n", p=P
                )
                nc.sync.dma_start(out=dst, in_=ot)
                return i_at

            offs = np.concatenate([[0], np.cumsum(MS)[:-1]])
            for cj in range(len(MS)):
                ot, r, xr, _ = stage1(cj, int(offs[cj]), MS[cj])
                stage2(cj, int(offs[cj]), MS[cj], ot, r, xr)

    _split_multi_waits(nc)
    return nc


_NC_CACHE = []


def _host_pack(mat_batch: np.ndarray) -> np.ndarray:
    """[B,3,3] -> [NCORES, N_C*9] tile-major/component-major PERM layout."""
    flat = np.ascontiguousarray(mat_batch, dtype=np.float32).reshape(
        NCORES, N_C, 9
    )
    out = np.empty((NCORES, N_C * 9), np.float32)
    pos = 0
    for m, off in zip(MS, np.concatenate([[0], np.cumsum(MS)[:-1]])):
        off = int(off)
        # chunk: matrices [off*P, (off+m)*P) viewed [P, m, 9] ->  [P, 9, m]
        chunk = flat[:, off * P : (off + m) * P, :].reshape(NCORES, P, m, 9)
        sz = P * m * 9
        out[:, pos : pos + sz] = (
            chunk.transpose(0, 1, 3, 2)[:, :, PERM, :].reshape(NCORES, sz)
        )
        pos += sz
    return out


def _host_unpack(res_list) -> np.ndarray:
    out = np.empty((B, 3), np.float32)
    o = out.reshape(NCORES, N_C, 3)
    for i, r in enumerate(res_list):
        y = r["twist_out"]
        pos = 0
        for m, off in zip(MS, np.concatenate([[0], np.cumsum(MS)[:-1]])):
            off = int(off)
            sz = P * m * 3
            blk = y[pos : pos + sz].reshape(P, 3, m)
            o[i, off * P : (off + m) * P, :] = blk.transpose(0, 2, 1).reshape(
                P * m, 3
            )
            pos += sz
    return out


def kernel(mat_batch: np.ndarray) -> np.ndarray:
    if not _NC_CACHE:
        _NC_CACHE.append(_build_kernel())
    nc = _NC_CACHE[0]

    packed = _host_pack(mat_batch)
    in_maps = [{"mat_in": packed[i]} for i in range(NCORES)]
    res = run_bass_kernel_spmd(nc, in_maps, core_ids=list(range(NCORES)))
    return _host_unpack(res.results)
